# revision 3
# baseline (speedup 1.0000x reference)
"""DynamicGate MoE routing kernel for Trainium2 (8 NeuronCores, Bass/Tile).

Computes, for x[N,H], sim_matrix[H,E], gates[E]:
    logits = l2norm_rows(x) @ l2norm_cols(sim_matrix)
    thr    = sigmoid(gates)
    gated  = relu(logits - thr)
    mask   = (gated > 0), with top-1 fallback for all-inactive tokens
    probs  = softmax over active experts of gated
Returns (mask, probs, logits), all [N, E] fp32.

Sharding: data-parallel on the token dim across 8 cores (2048 tokens per
core); sim_matrix/gates replicated. No collectives needed.
"""

import sys

if "/opt/trn_rl_repo" not in sys.path:
    sys.path.insert(0, "/opt/trn_rl_repo")

import numpy as np

import concourse.bacc as bacc
import concourse.mybir as mybir
from concourse import bass_utils, masks
from concourse.tile import TileContext

F32 = mybir.dt.float32
OP = mybir.AluOpType
AF = mybir.ActivationFunctionType
AX = mybir.AxisListType

N, H, E = 16384, 2048, 64
NCORES = 8
NLOC = N // NCORES   # 2048 tokens per core
PB = 128             # tokens per block (partition dim)
HC = H // 128        # 16 h-chunks
EPS = 1e-12


def build(repeat=1, nblk=NLOC // PB):
    nc = bacc.Bacc("TRN2", target_bir_lowering=False, debug=False)
    x_d = nc.dram_tensor("x", [NLOC, H], F32, kind="ExternalInput")
    sim_d = nc.dram_tensor("sim", [H, E], F32, kind="ExternalInput")
    gates_d = nc.dram_tensor("gates", [1, E], F32, kind="ExternalInput")
    mask_d = nc.dram_tensor("mask", [NLOC, E], F32, kind="ExternalOutput")
    probs_d = nc.dram_tensor("probs", [NLOC, E], F32, kind="ExternalOutput")
    logits_d = nc.dram_tensor("logits", [NLOC, E], F32, kind="ExternalOutput")

    with TileContext(nc) as tc:
        with (
            tc.tile_pool(name="const", bufs=1) as constp,
            tc.tile_pool(name="xin", bufs=3) as xinp,
            tc.tile_pool(name="xt", bufs=2) as xtp,
            tc.tile_pool(name="sq", bufs=2) as sqp,
            tc.tile_pool(name="ep", bufs=3) as epp,
            tc.tile_pool(name="sc", bufs=4) as scp,
            tc.tile_pool(name="psA", bufs=2, space="PSUM") as psA,
            tc.tile_pool(name="psB", bufs=2, space="PSUM") as psB,
            tc.tile_pool(name="psC", bufs=1, space="PSUM") as psC,
        ):
            # ---- preamble: constants -------------------------------------
            ident = constp.tile([128, 128], F32, name="ident")
            masks.make_identity(nc, ident)
            onesc = constp.tile([128, 1], F32, name="onesc")
            nc.gpsimd.memset(onesc, 1.0)
            onesr = constp.tile([1, 128], F32, name="onesr")
            nc.gpsimd.memset(onesr, 1.0)

            # sim_matrix as 16 chunks [h=128, E] side by side: wn[:, c*E:(c+1)*E]
            wn = constp.tile([128, HC * E], F32, name="wn")
            nc.sync.dma_start(
                out=wn.rearrange("p (c e) -> p c e", e=E),
                in_=sim_d.ap().rearrange("(c p) e -> p c e", p=128),
            )
            # column sumsq of sim via ACT square + PE ones-matmul
            wnsq = constp.tile([128, HC * E], F32, name="wnsq")
            nc.scalar.square(wnsq, wn)
            cs_ps = psC.tile([1, E], F32, name="cs_ps")
            for c in range(HC):
                nc.tensor.matmul(
                    cs_ps, lhsT=onesc, rhs=wnsq[:, c * E:(c + 1) * E],
                    start=(c == 0), stop=(c == HC - 1),
                )
            wnorm = constp.tile([1, E], F32, name="wnorm")
            nc.scalar.sqrt(wnorm, cs_ps)
            nc.vector.tensor_scalar(
                out=wnorm, in0=wnorm, scalar1=EPS, scalar2=None, op0=OP.max
            )
            rwn = constp.tile([1, E], F32, name="rwn")
            nc.vector.reciprocal(rwn, wnorm)

            g_row = constp.tile([1, E], F32, name="g_row")
            nc.sync.dma_start(out=g_row, in_=gates_d.ap())
            thr_row = constp.tile([1, E], F32, name="thr_row")
            nc.scalar.activation(thr_row, g_row, AF.Sigmoid)

            # broadcast [1,E] rows across 128 partitions via rank-1 matmul
            bc_ps = psC.tile([128, 2 * E], F32, name="bc_ps")
            nc.tensor.matmul(bc_ps[:, 0:E], lhsT=onesr, rhs=rwn,
                             start=True, stop=True)
            nc.tensor.matmul(bc_ps[:, E:2 * E], lhsT=onesr, rhs=thr_row,
                             start=True, stop=True)
            rwn_b = constp.tile([128, E], F32, name="rwn_b")
            thr_b = constp.tile([128, E], F32, name="thr_b")
            nc.scalar.copy(rwn_b, bc_ps[:, 0:E])
            nc.scalar.copy(thr_b, bc_ps[:, E:2 * E])

            # ---- main loop over token blocks -----------------------------
            for r in range(repeat):
                for b in range(nblk):
                    tok = slice(b * PB, (b + 1) * PB)
                    x_nat = xinp.tile([128, H], F32, name="x_nat", tag="x_nat")
                    nc.sync.dma_start(out=x_nat, in_=x_d.ap()[tok, :])

                    # row sumsq -> 1/max(||x||, eps)
                    sq = sqp.tile([128, H], F32, name="sq", tag="sq")
                    ssq = scp.tile([128, 1], F32, name="ssq", tag="ssq")
                    nc.vector.scalar_tensor_tensor(
                        out=sq, in0=x_nat, scalar=1.0, in1=x_nat,
                        op0=OP.mult, op1=OP.mult, accum_out=ssq,
                    )
                    nrm = scp.tile([128, 1], F32, name="nrm", tag="nrm")
                    nc.scalar.sqrt(nrm, ssq)
                    nc.vector.tensor_scalar(
                        out=nrm, in0=nrm, scalar1=EPS, scalar2=None, op0=OP.max
                    )
                    rnorm = scp.tile([128, 1], F32, name="rnorm", tag="rnorm")
                    nc.vector.reciprocal(rnorm, nrm)

                    # transpose x block: 4 groups of 4 chunks -> xt [h, tok]
                    xt = xtp.tile([128, H], F32, name="xt", tag="xt")
                    for g in range(4):
                        pt = psA.tile([128, 512], F32, name="pt", tag="pt")
                        for j in range(4):
                            c = 4 * g + j
                            nc.tensor.transpose(
                                pt[:, j * 128:(j + 1) * 128],
                                x_nat[:, c * 128:(c + 1) * 128],
                                ident,
                            )
                        nc.scalar.copy(xt[:, g * 512:(g + 1) * 512], pt)

                    # logits[tok, E] += xt_c.T @ wn_c over 16 chunks
                    pl = psB.tile([128, E], F32, name="pl", tag="pl")
                    for c in range(HC):
                        nc.tensor.matmul(
                            pl,
                            lhsT=xt[:, c * 128:(c + 1) * 128],
                            rhs=wn[:, c * E:(c + 1) * E],
                            start=(c == 0), stop=(c == HC - 1),
                        )

                    # epilogue
                    logits = epp.tile([128, E], F32, name="logits", tag="logits")
                    nc.vector.scalar_tensor_tensor(
                        out=logits, in0=pl, scalar=rnorm, in1=rwn_b,
                        op0=OP.mult, op1=OP.mult,
                    )
                    gsub = epp.tile([128, E], F32, name="gsub", tag="gsub")
                    nc.vector.tensor_tensor(
                        out=gsub, in0=logits, in1=thr_b, op=OP.subtract
                    )
                    gated = epp.tile([128, E], F32, name="gated", tag="gated")
                    nc.scalar.activation(gated, gsub, AF.Relu)
                    ind = epp.tile([128, E], F32, name="ind", tag="ind")
                    nact = scp.tile([128, 1], F32, name="nact", tag="nact")
                    nc.scalar.activation(ind, gated, AF.Sign, accum_out=nact)
                    inact = scp.tile([128, 1], F32, name="inact", tag="inact")
                    nc.vector.tensor_scalar(
                        out=inact, in0=nact, scalar1=0.0, scalar2=None,
                        op0=OP.is_equal,
                    )
                    lmax = scp.tile([128, 1], F32, name="lmax", tag="lmax")
                    nc.vector.tensor_reduce(
                        out=lmax, in_=logits, axis=AX.X, op=OP.max
                    )
                    onehot = epp.tile([128, E], F32, name="onehot", tag="onehot")
                    nc.vector.tensor_scalar(
                        out=onehot, in0=logits, scalar1=lmax, scalar2=None,
                        op0=OP.is_equal,
                    )
                    maskt = epp.tile([128, E], F32, name="maskt", tag="maskt")
                    nc.vector.scalar_tensor_tensor(
                        out=maskt, in0=onehot, scalar=inact, in1=ind,
                        op0=OP.mult, op1=OP.add,
                    )
                    ngmax = scp.tile([128, 1], F32, name="ngmax", tag="ngmax")
                    nc.vector.tensor_reduce(
                        out=ngmax, in_=gated, axis=AX.X, op=OP.max, negate=True
                    )
                    ex = epp.tile([128, E], F32, name="ex", tag="ex")
                    nc.scalar.activation(ex, gated, AF.Exp, bias=ngmax)
                    me = epp.tile([128, E], F32, name="me", tag="me")
                    sesum = scp.tile([128, 1], F32, name="sesum", tag="sesum")
                    nc.vector.scalar_tensor_tensor(
                        out=me, in0=ex, scalar=1.0, in1=maskt,
                        op0=OP.mult, op1=OP.mult, accum_out=sesum,
                    )
                    rs = scp.tile([128, 1], F32, name="rs", tag="rs")
                    nc.vector.reciprocal(rs, sesum)
                    probs = epp.tile([128, E], F32, name="probs", tag="probs")
                    nc.scalar.activation(probs, me, AF.Copy, scale=rs)

                    nc.sync.dma_start(out=mask_d.ap()[tok, :], in_=maskt)
                    nc.sync.dma_start(out=probs_d.ap()[tok, :], in_=probs)
                    nc.sync.dma_start(out=logits_d.ap()[tok, :], in_=logits)

    nc.compile()
    return nc


_NC_CACHE = {}


def _get_nc(repeat=1, nblk=NLOC // PB):
    key = (repeat, nblk)
    if key not in _NC_CACHE:
        _NC_CACHE[key] = build(repeat, nblk)
    return _NC_CACHE[key]


def make_in_maps(x, sim_matrix, gates):
    x = np.ascontiguousarray(np.asarray(x, dtype=np.float32))
    sim = np.ascontiguousarray(np.asarray(sim_matrix, dtype=np.float32))
    g = np.ascontiguousarray(np.asarray(gates, dtype=np.float32)).reshape(1, E)
    return [
        {"x": x[c * NLOC:(c + 1) * NLOC], "sim": sim, "gates": g}
        for c in range(NCORES)
    ]


def kernel(x, sim_matrix, gates):
    nc = _get_nc()
    in_maps = make_in_maps(x, sim_matrix, gates)
    res = bass_utils.run_bass_kernel_spmd(nc, in_maps, core_ids=list(range(NCORES)))
    mask = np.concatenate([res.results[c]["mask"] for c in range(NCORES)], axis=0)
    probs = np.concatenate([res.results[c]["probs"] for c in range(NCORES)], axis=0)
    logits = np.concatenate([res.results[c]["logits"] for c in range(NCORES)], axis=0)
    return mask, probs, logits


# revision 20
# speedup vs baseline: 5.5430x; 5.5430x over previous
"""DynamicGate MoE routing kernel for Trainium2 (8 NeuronCores, Bass/Tile).

Computes, for x[N,H], sim_matrix[H,E], gates[E]:
    logits = l2norm_rows(x) @ l2norm_cols(sim_matrix)
    thr    = sigmoid(gates)
    gated  = relu(logits - thr)
    mask   = (gated > 0), with top-1 fallback for all-inactive tokens
    probs  = softmax over active experts of gated
Returns (mask, probs, logits), all [N, E] fp32.

Sharding: data-parallel on the token dim across 8 cores (2048 tokens per
core); sim_matrix/gates replicated. No collectives needed.

Per-core dataflow (16 token blocks of 128):
  - DMA x block [128, 2048] in natural layout
  - DVE fused square+accum for row sumsq
  - PE transposes x chunks (fp32, identity streaming) into PSUM;
    ACT/DVE copy them to SBUF
  - 16 accumulating fp32 matmuls logits[tok,E] += xT_c.T @ wn_c, kept
    resident in PSUM (two 8-block group tiles, 1 bank each)
  - per 8-block group: batched sqrt+reciprocal of the row norms
  - epilogue per block straight from PSUM: scale, threshold, STE mask,
    top-1 fallback (equality vs row max), masked softmax
"""

import sys

if "/opt/trn_rl_repo" not in sys.path:
    sys.path.insert(0, "/opt/trn_rl_repo")

import numpy as np

import concourse.bacc as bacc
import concourse.mybir as mybir
from concourse import bass_utils, masks
from concourse.tile import TileContext

F32 = mybir.dt.float32
OP = mybir.AluOpType
AF = mybir.ActivationFunctionType
AX = mybir.AxisListType

N, H, E = 16384, 2048, 64
NCORES = 8
NLOC = N // NCORES   # 2048 tokens per core
PB = 128             # tokens per block (partition dim)
HC = H // 128        # 16 h-chunks
GRP = 4              # blocks per rsqrt/psum group
EPS = 1e-12


def build(repeat=1, nblk=NLOC // PB):
    nc = bacc.Bacc("TRN2", target_bir_lowering=False, debug=False)
    x_d = nc.dram_tensor("x", [NLOC, H], F32, kind="ExternalInput")
    sim_d = nc.dram_tensor("sim", [H, E], F32, kind="ExternalInput")
    gates_d = nc.dram_tensor("gates", [1, E], F32, kind="ExternalInput")
    mask_d = nc.dram_tensor("mask", [NLOC, E], F32, kind="ExternalOutput")
    probs_d = nc.dram_tensor("probs", [NLOC, E], F32, kind="ExternalOutput")
    logits_d = nc.dram_tensor("logits", [NLOC, E], F32, kind="ExternalOutput")

    ngrp = (nblk + GRP - 1) // GRP

    with TileContext(nc) as tc:
        with (
            tc.tile_pool(name="const", bufs=1) as constp,
            tc.tile_pool(name="xin", bufs=4) as xinp,
            tc.tile_pool(name="xt", bufs=3) as xtp,
            tc.tile_pool(name="sq", bufs=2) as sqp,
            tc.tile_pool(name="ep", bufs=3) as epp,
            tc.tile_pool(name="sc", bufs=4) as scp,
            tc.tile_pool(name="grp", bufs=2) as grpp,
            tc.tile_pool(name="psA", bufs=2, space="PSUM") as psA,
            tc.tile_pool(name="psB", bufs=2, space="PSUM") as psB,
            tc.tile_pool(name="psC", bufs=1, space="PSUM") as psC,
        ):
            # ---- preamble: constants -------------------------------------
            ident = constp.tile([128, 128], F32, name="ident")
            masks.make_identity(nc, ident)
            onesc = constp.tile([128, 1], F32, name="onesc")
            nc.gpsimd.memset(onesc, 1.0)
            onesr = constp.tile([1, 128], F32, name="onesr")
            nc.gpsimd.memset(onesr, 1.0)

            # sim_matrix as 16 chunks [h=128, E] side by side; its DMA is
            # emitted after the first x prefetches (x gates the PE sooner)
            wn = constp.tile([128, HC * E], F32, name="wn")
            g_row = constp.tile([1, E], F32, name="g_row")

            def emit_const_dmas():
                nc.sync.dma_start(
                    out=wn.rearrange("p (c e) -> p c e", e=E),
                    in_=sim_d.ap().rearrange("(c p) e -> p c e", p=128),
                )
                nc.sync.dma_start(out=g_row, in_=gates_d.ap())

            rwn_b = constp.tile([128, E], F32, name="rwn_b")
            thr_b = constp.tile([128, E], F32, name="thr_b")

            def emit_wn_norm_preamble():
                # column sumsq of sim via ACT square + PE ones-matmul;
                # emitted after the first group's block work so the PE
                # starts on transposes immediately
                wnsq = constp.tile([128, HC * E], F32, name="wnsq")
                nc.scalar.square(wnsq, wn)
                cs_ps = psC.tile([1, E], F32, name="cs_ps", tag="cs")
                for c in range(HC):
                    nc.tensor.matmul(
                        cs_ps, lhsT=onesc, rhs=wnsq[:, c * E:(c + 1) * E],
                        start=(c == 0), stop=(c == HC - 1),
                    )
                wnorm = constp.tile([1, E], F32, name="wnorm")
                nc.scalar.sqrt(wnorm, cs_ps)
                nc.vector.tensor_scalar(
                    out=wnorm, in0=wnorm, scalar1=EPS, scalar2=None, op0=OP.max
                )
                rwn = constp.tile([1, E], F32, name="rwn")
                nc.vector.reciprocal(rwn, wnorm)

                # thr = sigmoid(g) = 1/(1+exp(-g))
                eneg = constp.tile([1, E], F32, name="eneg")
                nc.scalar.activation(eneg, g_row, AF.Exp, scale=-1.0)
                nc.vector.tensor_scalar(
                    out=eneg, in0=eneg, scalar1=1.0, scalar2=None, op0=OP.add
                )
                thr_row = constp.tile([1, E], F32, name="thr_row")
                nc.vector.reciprocal(thr_row, eneg)

                # broadcast [1,E] rows across 128 partitions via rank-1 matmul
                bc_ps = psC.tile([128, 2 * E], F32, name="bc_ps", tag="bc")
                nc.tensor.matmul(bc_ps[:, 0:E], lhsT=onesr, rhs=rwn,
                                 start=True, stop=True)
                nc.tensor.matmul(bc_ps[:, E:2 * E], lhsT=onesr, rhs=thr_row,
                                 start=True, stop=True)
                nc.scalar.copy(rwn_b, bc_ps[:, 0:E])
                nc.scalar.copy(thr_b, bc_ps[:, E:2 * E])

            # ---- main loop: groups of GRP token blocks -------------------
            first_emit = True
            for r in range(repeat):
                x_tiles = {}

                def prefetch(b):
                    t = xinp.tile([128, H], F32, name="x_nat", tag="x_nat")
                    nc.sync.dma_start(
                        out=t, in_=x_d.ap()[b * PB:(b + 1) * PB, :]
                    )
                    x_tiles[b] = t

                prefetch(0)
                if nblk > 1:
                    prefetch(1)
                if r == 0:
                    emit_const_dmas()

                for g in range(ngrp):
                    blo = g * GRP
                    bhi = min(blo + GRP, nblk)
                    nb = bhi - blo
                    # group-resident logits accumulators (1 PSUM bank)
                    plg = psB.tile([128, GRP, E], F32, name="plg", tag="plg")
                    ssqg = grpp.tile([128, GRP], F32, name="ssqg", tag="ssqg")

                    for j in range(nb):
                        b = blo + j
                        tok = slice(b * PB, (b + 1) * PB)
                        x_nat = x_tiles.pop(b)
                        if b + 2 < nblk:
                            prefetch(b + 2)

                        # row sumsq into group column j
                        sq = sqp.tile([128, H], F32, name="sq", tag="sq")
                        nc.vector.scalar_tensor_tensor(
                            out=sq, in0=x_nat, scalar=1.0, in1=x_nat,
                            op0=OP.mult, op1=OP.mult,
                            accum_out=ssqg[:, j:j + 1],
                        )

                        # transpose x block: 2 half-groups of 8 chunks
                        xt = xtp.tile([128, H], F32, name="xt", tag="xt")
                        for hf in range(2):
                            pt = psA.tile([128, 1024], F32, name="pt", tag="pt")
                            for k in range(8):
                                c = 8 * hf + k
                                nc.tensor.transpose(
                                    pt[:, k * 128:(k + 1) * 128],
                                    x_nat[:, c * 128:(c + 1) * 128],
                                    ident,
                                )
                            nc.scalar.copy(
                                xt[:, hf * 1024:(hf + 1) * 1024], pt
                            )

                        # logits[tok, E] += xt_c.T @ wn_c
                        for c in range(HC):
                            nc.tensor.matmul(
                                plg[:, j, :],
                                lhsT=xt[:, c * 128:(c + 1) * 128],
                                rhs=wn[:, c * E:(c + 1) * E],
                                start=(c == 0), stop=(c == HC - 1),
                            )

                    if first_emit:
                        emit_wn_norm_preamble()
                        first_emit = False

                    # batched 1/max(||x||,eps) for the group
                    nrmg = grpp.tile([128, GRP], F32, name="nrmg", tag="nrmg")
                    nc.scalar.sqrt(nrmg[:, 0:nb], ssqg[:, 0:nb])
                    nc.vector.tensor_scalar(
                        out=nrmg[:, 0:nb], in0=nrmg[:, 0:nb], scalar1=EPS,
                        scalar2=None, op0=OP.max,
                    )
                    rng = grpp.tile([128, GRP], F32, name="rng", tag="rng")
                    nc.vector.reciprocal(rng[:, 0:nb], nrmg[:, 0:nb])

                    # ---- group-wide epilogue on [128, nb, E] tiles -------
                    def bce(ap):   # [128, nb] -> [128, nb, E] stride-0
                        return ap.unsqueeze(2).broadcast_to([128, nb, E])

                    pls = plg[:, 0:nb, :]
                    scl = epp.tile([128, GRP, E], F32, name="scl", tag="scl")
                    nc.vector.tensor_tensor(
                        out=scl[:, 0:nb, :], in0=bce(rng[:, 0:nb]),
                        in1=rwn_b[:, :].unsqueeze(1).broadcast_to([128, nb, E]),
                        op=OP.mult,
                    )
                    logits = epp.tile([128, GRP, E], F32, name="logits", tag="logits")
                    nc.vector.tensor_tensor(
                        out=logits[:, 0:nb, :], in0=pls, in1=scl[:, 0:nb, :],
                        op=OP.mult,
                    )
                    gsub = epp.tile([128, GRP, E], F32, name="gsub", tag="gsub")
                    nc.vector.tensor_tensor(
                        out=gsub[:, 0:nb, :], in0=logits[:, 0:nb, :],
                        in1=thr_b[:, :].unsqueeze(1).broadcast_to([128, nb, E]),
                        op=OP.subtract,
                    )
                    gated = epp.tile([128, GRP, E], F32, name="gated", tag="gated")
                    nc.vector.tensor_scalar(
                        out=gated[:, 0:nb, :], in0=gsub[:, 0:nb, :],
                        scalar1=0.0, scalar2=None, op0=OP.max,
                    )
                    ind = epp.tile([128, GRP, E], F32, name="ind", tag="ind")
                    nc.vector.tensor_scalar(
                        out=ind[:, 0:nb, :], in0=gsub[:, 0:nb, :],
                        scalar1=0.0, scalar2=None, op0=OP.is_gt,
                    )
                    nact = grpp.tile([128, GRP], F32, name="nact", tag="nact")
                    nc.vector.tensor_reduce(
                        out=nact[:, 0:nb], in_=ind[:, 0:nb, :], axis=AX.X,
                        op=OP.add,
                    )
                    inact = grpp.tile([128, GRP], F32, name="inact", tag="inact")
                    nc.vector.tensor_scalar(
                        out=inact[:, 0:nb], in0=nact[:, 0:nb], scalar1=0.0,
                        scalar2=None, op0=OP.is_equal,
                    )
                    lmax = grpp.tile([128, GRP], F32, name="lmax", tag="lmax")
                    nc.vector.tensor_reduce(
                        out=lmax[:, 0:nb], in_=logits[:, 0:nb, :], axis=AX.X,
                        op=OP.max,
                    )
                    onehot = epp.tile([128, GRP, E], F32, name="onehot", tag="onehot")
                    nc.vector.tensor_tensor(
                        out=onehot[:, 0:nb, :], in0=logits[:, 0:nb, :],
                        in1=bce(lmax[:, 0:nb]), op=OP.is_equal,
                    )
                    maskt = epp.tile([128, GRP, E], F32, name="maskt", tag="maskt")
                    nc.vector.tensor_tensor(
                        out=maskt[:, 0:nb, :], in0=onehot[:, 0:nb, :],
                        in1=bce(inact[:, 0:nb]), op=OP.mult,
                    )
                    nc.vector.tensor_tensor(
                        out=maskt[:, 0:nb, :], in0=maskt[:, 0:nb, :],
                        in1=ind[:, 0:nb, :], op=OP.add,
                    )
                    gmax = grpp.tile([128, GRP], F32, name="gmax", tag="gmax")
                    nc.vector.tensor_reduce(
                        out=gmax[:, 0:nb], in_=gated[:, 0:nb, :], axis=AX.X,
                        op=OP.max,
                    )
                    gsh = epp.tile([128, GRP, E], F32, name="gsh", tag="gsh")
                    nc.vector.tensor_tensor(
                        out=gsh[:, 0:nb, :], in0=gated[:, 0:nb, :],
                        in1=bce(gmax[:, 0:nb]), op=OP.subtract,
                    )
                    ex = epp.tile([128, GRP, E], F32, name="ex", tag="ex")
                    nc.scalar.activation(ex[:, 0:nb, :], gsh[:, 0:nb, :], AF.Exp)
                    me = epp.tile([128, GRP, E], F32, name="me", tag="me")
                    nc.vector.tensor_tensor(
                        out=me[:, 0:nb, :], in0=ex[:, 0:nb, :],
                        in1=maskt[:, 0:nb, :], op=OP.mult,
                    )
                    sesum = grpp.tile([128, GRP], F32, name="sesum", tag="sesum")
                    nc.vector.tensor_reduce(
                        out=sesum[:, 0:nb], in_=me[:, 0:nb, :], axis=AX.X,
                        op=OP.add,
                    )
                    rs = grpp.tile([128, GRP], F32, name="rs", tag="rs")
                    nc.vector.reciprocal(rs[:, 0:nb], sesum[:, 0:nb])
                    probs = epp.tile([128, GRP, E], F32, name="probs", tag="probs")
                    nc.vector.tensor_tensor(
                        out=probs[:, 0:nb, :], in0=me[:, 0:nb, :],
                        in1=bce(rs[:, 0:nb]), op=OP.mult,
                    )

                    gtok = slice(blo * PB, bhi * PB)
                    for out_d, src in ((mask_d, maskt), (probs_d, probs),
                                       (logits_d, logits)):
                        nc.sync.dma_start(
                            out=out_d.ap()[gtok, :].rearrange(
                                "(j p) e -> p j e", p=128),
                            in_=src[:, 0:nb, :],
                        )

    nc.compile()
    return nc


_NC_CACHE = {}


def _get_nc(repeat=1, nblk=NLOC // PB):
    key = (repeat, nblk)
    if key not in _NC_CACHE:
        _NC_CACHE[key] = build(repeat, nblk)
    return _NC_CACHE[key]


def make_in_maps(x, sim_matrix, gates):
    x = np.ascontiguousarray(np.asarray(x, dtype=np.float32))
    sim = np.ascontiguousarray(np.asarray(sim_matrix, dtype=np.float32))
    g = np.ascontiguousarray(np.asarray(gates, dtype=np.float32)).reshape(1, E)
    return [
        {"x": x[c * NLOC:(c + 1) * NLOC], "sim": sim, "gates": g}
        for c in range(NCORES)
    ]


def kernel(x, sim_matrix, gates):
    nc = _get_nc()
    in_maps = make_in_maps(x, sim_matrix, gates)
    res = bass_utils.run_bass_kernel_spmd(nc, in_maps, core_ids=list(range(NCORES)))
    mask = np.concatenate([res.results[c]["mask"] for c in range(NCORES)], axis=0)
    probs = np.concatenate([res.results[c]["probs"] for c in range(NCORES)], axis=0)
    logits = np.concatenate([res.results[c]["logits"] for c in range(NCORES)], axis=0)
    return mask, probs, logits
